# revision 1
# baseline (speedup 1.0000x reference)
# Trainium2 Bass kernel for nn_Attention: out = softmax(x @ (y@W + b) + mask*-1e9) @ x
# Sharding: data-parallel over batch, 1 batch element per NeuronCore (8 cores).
#
# Per-core math (S = D = 1024):
#   logits = x @ (y @ W) + rowsum(x) (x) b + mask * -1e9      [reassociated: (x@y)@W]
#   out    = softmax(logits) @ x
#
# Precision strategy: the logit chain runs as hi/lo-split fp16 matmuls (3 passes
# per matmul, fp16 products are exact in fp32 PSUM), giving near-fp32 logits at
# bf16-tier speed. The output matmul runs as a single fp16 pass. Inputs are
# pre-scaled (x,y by 16, W by 1024) so the fp16 "lo" residuals stay in normal
# fp16 range; the resulting 2^18 logit scale is folded into the exp affine.
#
# x is transposed on the tensor engine per 128x128 chunk (fine-grained deps so
# the PE ramps up immediately); the softmax output is transposed via DMA xbar.
import sys

import numpy as np

for _p in ("/opt/trn_rl_repo",):
    if _p not in sys.path:
        sys.path.insert(0, _p)

import concourse.bass as bass
from concourse import bacc
import concourse.mybir as mybir
import concourse.tile as tile
from concourse.bass_utils import run_bass_kernel_spmd

F32 = mybir.dt.float32
F16 = mybir.dt.float16

P = 128
FD = 512  # matmul moving free dim (one fp32 PSUM bank)

SX = 16.0  # x / y pre-scale
SW = 1024.0  # W pre-scale
SLOG = SX * SX * SW  # net logit scale = 2**18
MASKC = -1.0e9 * SLOG

NPASS = 3  # split passes per logit matmul (3 = drop lo*lo only; 2 = drop hi*lo too)

ALU = mybir.AluOpType
ACTF = mybir.ActivationFunctionType
AXIS = mybir.AxisListType


def build_nc(n=1024):
    """Build the per-core Bass program (SPMD: same program on all 8 cores)."""
    NT = n // P  # 128-tiles per dim
    NH = n // FD  # 512-halves per dim
    HC = NT // NH  # transposed chunks per half (4)

    nc = bacc.Bacc("TRN2", target_bir_lowering=False, debug=False)
    x_d = nc.dram_tensor("x", [n, n], F32, kind="ExternalInput")
    y_d = nc.dram_tensor("y", [n, n], F32, kind="ExternalInput")
    mask_d = nc.dram_tensor("mask", [n, n], F32, kind="ExternalInput")
    w_d = nc.dram_tensor("W", [n, n], F32, kind="ExternalInput")
    b_d = nc.dram_tensor("bvec", [1, n], F32, kind="ExternalInput")
    id_d = nc.dram_tensor("ident", [P, P], F32, kind="ExternalInput")
    out_d = nc.dram_tensor("out", [n, n], F32, kind="ExternalOutput")

    with tile.TileContext(nc) as tc:
        import contextlib

        ctx = contextlib.ExitStack()
        with ctx:
            persist = ctx.enter_context(tc.tile_pool(name="persist", bufs=1))
            ld = ctx.enter_context(tc.tile_pool(name="ld", bufs=3))
            epi = ctx.enter_context(tc.tile_pool(name="epi", bufs=2))
            small = ctx.enter_context(tc.tile_pool(name="small", bufs=4))
            psum = ctx.enter_context(tc.tile_pool(name="psum", bufs=7, space="PSUM"))
            psum_r = ctx.enter_context(
                tc.tile_pool(name="psum_r", bufs=1, space="PSUM")
            )
            dram = ctx.enter_context(
                tc.tile_pool(name="dram", bufs=1, space="DRAM")
            )

            # ---- persistent tensors (fp16 slabs are [P, NT, n] = 16KB/part) --
            x_hi = persist.tile([P, NT, n], F16, tag="x_hi")
            y_hi = persist.tile([P, NT, n], F16, tag="y_hi")
            y_lo = persist.tile([P, NT, n], F16, tag="slotD")  # reused for w_lo
            gt_hi = persist.tile([P, NT, n], F16, tag="gt_hi")
            gt_lo = persist.tile([P, NT, n], F16, tag="gt_lo")
            # transposed x, one slab per s-half: [P, kt, hc, P]
            xTh = [
                persist.tile(
                    [P, NT, HC, P], F16, tag=f"xTh_{h}", name=f"xTh_{h}"
                )
                for h in range(NH)
            ]
            xTl = None
            if NPASS >= 3:
                xTl = [
                    persist.tile(
                        [P, NT, HC, P], F16, tag=f"xTl_{h}", name=f"xTl_{h}"
                    )
                    for h in range(NH)
                ]

            ident = persist.tile([P, P], F32, tag="ident")
            nc.sync.dma_start(ident, id_d[:, :])
            # HAM warm-up: ~4.3us of dummy matmuls so the PE clock is at 8/8
            # before the first real transpose/matmul arrives.
            scratch = persist.tile([P, FD], F16, tag="scratch")
            nc.gpsimd.memset(scratch, 0.0)
            # fp32 ident@ident: depends only on the first sync-queue DMA, so
            # the PE clock is warming by ~2us
            wps = psum_r.tile([P, FD], F32, tag="rsx", name="warm_ps")
            for i in range(7):
                nc.tensor.matmul(
                    wps[:, 0:P], lhsT=ident, rhs=ident, start=(i == 0), stop=(i == 6)
                )
            rxs = persist.tile([P, NT], F32, tag="rxs")
            recip = [
                persist.tile([P, 1], F32, tag=f"recip{i}", name=f"recip{i}")
                for i in range(NT)
            ]
            et = [
                [
                    persist.tile(
                        [P, HC, P], F16, tag=f"et{i}_{h}", name=f"et{i}_{h}"
                    )
                    for h in range(NH)
                ]
                for i in range(NT)
            ]

            # ---- stage 0: load x, split, transpose chunks on PE -------------
            # x tiles 4..7 (only needed for the second g half) load after y so
            # y gets full DMA bandwidth during the ramp.
            def x_tile(it):
                xt = ld.tile([P, n], F32, tag="ld", name=f"xt{it}")
                nc.sync.dma_start(xt, x_d[P * it : P * (it + 1), :])
                # exact power-of-two pre-scale so every downstream op is a copy
                nc.vector.tensor_scalar_mul(xt, xt, SX)
                nc.vector.tensor_copy(x_hi[:, it, :], xt)
                h, hc = it // HC, it % HC
                for cb in range(NT // HC):
                    ptb = psum.tile(
                        [P, HC, P], F32, tag="mm", name=f"pt_{it}_{cb}"
                    )
                    for j in range(HC):
                        c = cb * HC + j
                        nc.tensor.transpose(
                            ptb[:, j, :], xt[:, P * c : P * (c + 1)], ident
                        )
                    dsth = xTh[h][:, cb * HC : (cb + 1) * HC, hc, :]
                    nc.vector.tensor_copy(dsth, ptb)
                    if NPASS >= 3:
                        nc.vector.scalar_tensor_tensor(
                            out=xTl[h][:, cb * HC : (cb + 1) * HC, hc, :],
                            in0=ptb,
                            scalar=1.0,
                            in1=dsth,
                            op0=ALU.mult,
                            op1=ALU.subtract,
                        )
                    hp = psum_r.tile(
                        [P, FD], F32, tag="rsx", name=f"ham_{it}_{cb}"
                    )
                    for i in range(2):
                        nc.tensor.matmul(
                            hp,
                            lhsT=scratch[:, 0:P],
                            rhs=scratch,
                            start=(i == 0),
                            stop=(i == 1),
                        )
                nc.vector.tensor_reduce(
                    rxs[:, it : it + 1], xt, axis=AXIS.X, op=ALU.add
                )

            for it in range(NT):
                x_tile(it)

            # ---- load y (SWDGE queue, concurrent with x on HWDGE), split ----
            for kt in range(NT):
                yt = ld.tile([P, n], F32, tag="ld")
                nc.gpsimd.dma_start(yt, y_d[P * kt : P * (kt + 1), :])
                nc.vector.tensor_scalar_mul(y_hi[:, kt, :], yt, SX)
                nc.vector.scalar_tensor_tensor(
                    out=y_lo[:, kt, :],
                    in0=yt,
                    scalar=SX,
                    in1=y_hi[:, kt, :],
                    op0=ALU.mult,
                    op1=ALU.subtract,
                )

            # bias term is added on DVE in the softmax epilogue:
            #   am += (SLOG/SX * b)[t] * (SX * rowsum_x)[s]
            # b is scaled fp32, broadcast across partitions via a DRAM bounce
            # (descriptor replication). Traced after the y loads so the
            # broadcast traffic does not delay the g-stage ramp.
            b_sb = persist.tile([1, n], F32, tag="b_sb")
            nc.gpsimd.dma_start(b_sb, b_d[:, :])
            nc.vector.tensor_scalar_mul(b_sb, b_sb, SLOG / SX)
            b_dr = dram.tile([1, n], F32, name="b_dr")
            nc.gpsimd.dma_start(b_dr[0:1, :], b_sb)
            b_bc = persist.tile([P, n], F32, tag="b_bc")
            bsrc = b_dr[0:1, :]
            nc.gpsimd.dma_start(
                b_bc[:, :],
                bass.AP(tensor=bsrc.tensor, offset=bsrc.offset,
                        ap=[[0, P], bsrc.ap[1]]),
            )

            # ---- g stage: gT[d, s] = SX^2 * sum_k x[s,k] y[k,d] -------------
            # sh-major so the first half only needs x tiles 0..HC-1 transposed.
            g_passes = [(y_hi, xTh), (y_lo, xTh)]
            if NPASS >= 3:
                g_passes.append((y_hi, xTl))
            nmm = NPASS * NT

            def g_epilogue(sh, dt, ps):
                nc.vector.tensor_copy(
                    gt_hi[:, dt, FD * sh : FD * (sh + 1)], ps
                )
                nc.vector.scalar_tensor_tensor(
                    out=gt_lo[:, dt, FD * sh : FD * (sh + 1)],
                    in0=ps,
                    scalar=1.0,
                    in1=gt_hi[:, dt, FD * sh : FD * (sh + 1)],
                    op0=ALU.mult,
                    op1=ALU.subtract,
                )

            def g_wavefront(sh, dts):
                # interleave groups across psum banks: each arriving y tile
                # unlocks len(dts)*NPASS matmuls so the PE FIFO never blocks
                # on a single group's kt ladder during the load phase
                pss = [
                    (dt, psum.tile([P, FD], F32, tag="mm", name=f"g{sh}_{dt}"))
                    for dt in dts
                ]
                cnt = dict.fromkeys(dts, 0)
                for kt in range(NT):
                    for dt, ps in pss:
                        for lhs, rhs in g_passes:
                            nc.tensor.matmul(
                                ps,
                                lhsT=lhs[:, kt, P * dt : P * (dt + 1)],
                                rhs=rhs[sh][:, kt, :, :],
                                start=(cnt[dt] == 0),
                                stop=(cnt[dt] == nmm - 1),
                            )
                            cnt[dt] += 1
                for dt, ps in pss:
                    g_epilogue(sh, dt, ps)

            def g_half(sh, wavefront=False):
                if wavefront:
                    g_wavefront(sh, list(range(4)))
                    rest = range(4, NT)
                else:
                    rest = range(NT)
                for dt in rest:
                    ps = psum.tile([P, FD], F32, tag="mm", name=f"g{sh}_{dt}")
                    idx = 0
                    for kt in range(NT):
                        for lhs, rhs in g_passes:
                            nc.tensor.matmul(
                                ps,
                                lhsT=lhs[:, kt, P * dt : P * (dt + 1)],
                                rhs=rhs[sh][:, kt, :, :],
                                start=(idx == 0),
                                stop=(idx == nmm - 1),
                            )
                            idx += 1
                    g_epilogue(sh, dt, ps)

            g_half(0, wavefront=True)
            for _sh in range(1, NH):
                g_half(_sh)

            # ---- load W (SWDGE), split (reuses x_lo / y_lo slots) -----------
            w_hi = persist.tile([P, NT, n], F16, tag="w_hi")
            w_lo = None
            if NPASS >= 3:
                w_lo = persist.tile([P, NT, n], F16, tag="w_lo", name="w_lo")
            for dt in range(NT):
                wt = ld.tile([P, n], F32, tag="ld")
                nc.gpsimd.dma_start(wt, w_d[P * dt : P * (dt + 1), :])
                nc.scalar.mul(w_hi[:, dt, :], wt, SW)
                if NPASS >= 3:
                    nc.vector.scalar_tensor_tensor(
                        out=w_lo[:, dt, :],
                        in0=wt,
                        scalar=SW,
                        in1=w_hi[:, dt, :],
                        op0=ALU.mult,
                        op1=ALU.subtract,
                    )

            # ---- a stage + softmax ------------------------------------------
            for st in range(NT):
                mk = ld.tile([P, n], F32, tag="ld")
                nc.sync.dma_start(mk, mask_d[P * st : P * (st + 1), :])
                am = epi.tile([P, n], F32, tag="am")
                for th in range(NH):
                    ps = psum.tile([P, FD], F32, tag="mm", name=f"a{st}_{th}")
                    idx = 0
                    a_passes = [(gt_hi, w_hi), (gt_lo, w_hi)]
                    if NPASS >= 3:
                        a_passes.append((gt_hi, w_lo))
                    for lhs, rhs in a_passes:
                        for dt in range(NT):
                            nc.tensor.matmul(
                                ps,
                                lhsT=lhs[:, dt, P * st : P * (st + 1)],
                                rhs=rhs[:, dt, FD * th : FD * (th + 1)],
                                start=(idx == 0),
                                stop=(idx == NPASS * NT - 1),
                            )
                            idx += 1
                    # masked scaled logits: am = mask*MASKC + psum
                    nc.vector.scalar_tensor_tensor(
                        out=am[:, FD * th : FD * (th + 1)],
                        in0=mk[:, FD * th : FD * (th + 1)],
                        scalar=MASKC,
                        in1=ps,
                        op0=ALU.mult,
                        op1=ALU.add,
                    )
                # am += b_bc[t] * rowsum_x[s]  (rank-1 bias on DVE)
                nc.vector.scalar_tensor_tensor(
                    out=am,
                    in0=b_bc,
                    scalar=rxs[:, st : st + 1],
                    in1=am,
                    op0=ALU.mult,
                    op1=ALU.add,
                )
                nm = small.tile([P, 1], F32, tag="nm")
                nc.vector.tensor_reduce(
                    nm, am, axis=AXIS.X, op=ALU.max, negate=True
                )
                nms = small.tile([P, 1], F32, tag="nms")
                nc.vector.tensor_scalar_mul(nms, nm, 1.0 / SLOG)
                eh = epi.tile([P, n], F16, tag="eh")
                rs = small.tile([P, 1], F32, tag="rs")
                nc.scalar.activation(
                    eh, am, ACTF.Exp, bias=nms, scale=1.0 / SLOG, accum_out=rs
                )
                nc.vector.reciprocal(recip[st], rs)
                for h in range(NH):
                    nc.scalar.dma_start_transpose(
                        et[st][h][:, :, :], eh[:, FD * h : FD * (h + 1)]
                    )

            # ---- out stage: out[s, e] = (e_hat @ x_hi) * recip / SX ---------
            for st in range(NT):
                # both halves interleaved: consecutive matmuls share lhsT and
                # group boundaries drop from every 8 matmuls to every 16
                opair = [
                    (h, psum.tile([P, FD], F32, tag="mm", name=f"o{st}_{h}"))
                    for h in range(NH)
                ]
                for tt in range(NT):
                    for h, ps in opair:
                        nc.tensor.matmul(
                            ps,
                            lhsT=et[st][tt // HC][:, tt % HC, :],
                            rhs=x_hi[:, tt, FD * h : FD * (h + 1)],
                            start=(tt == 0),
                            stop=(tt == NT - 1),
                        )
                for h, ps in opair:
                    ob = epi.tile([P, FD], F32, tag="ob")
                    nc.vector.tensor_scalar(
                        ob,
                        ps,
                        recip[st],
                        1.0 / SX,
                        ALU.mult,
                        ALU.mult,
                    )
                    nc.sync.dma_start(
                        out_d[P * st : P * (st + 1), FD * h : FD * (h + 1)], ob
                    )
    nc.compile()
    return nc


_NC_CACHE = {}


def _get_nc(n=1024):
    if n not in _NC_CACHE:
        _NC_CACHE[n] = build_nc(n)
    return _NC_CACHE[n]


def kernel(x, y, mask, W, b):
    """Full-input entry point: shard over batch across 8 cores, run, gather."""
    n = x.shape[-1]
    nc = _get_nc(n)
    Wc = np.ascontiguousarray(W, dtype=np.float32)
    bc = np.ascontiguousarray(np.asarray(b, dtype=np.float32).reshape(1, n))
    idc = np.eye(P, dtype=np.float32)
    in_maps = []
    for c in range(x.shape[0]):
        in_maps.append(
            {
                "x": np.ascontiguousarray(x[c], dtype=np.float32),
                "y": np.ascontiguousarray(y[c], dtype=np.float32),
                "mask": np.ascontiguousarray(mask[c], dtype=np.float32),
                "W": Wc,
                "bvec": bc,
                "ident": idc,
            }
        )
    res = run_bass_kernel_spmd(nc, in_maps, core_ids=list(range(len(in_maps))))
    return np.stack([r["out"] for r in res.results], axis=0)



# revision 7
# speedup vs baseline: 1.5906x; 1.5906x over previous
# Trainium2 Bass kernel for nn_Attention: out = softmax(x @ (y@W + b) + mask*-1e9) @ x
# Sharding: data-parallel over batch, 1 batch element per NeuronCore (8 cores).
#
# Per-core math (S = D = 1024):
#   logits = x @ (y @ W) + rowsum(x) (x) b + mask * -1e9      [reassociated: (x@y)@W]
#   out    = softmax(logits) @ x
#
# Precision strategy: the logit chain runs as hi/lo-split fp16 matmuls (3 passes
# per matmul, fp16 products are exact in fp32 PSUM), giving near-fp32 logits at
# bf16-tier speed. The output matmul runs as a single fp16 pass. Inputs are
# pre-scaled (x,y by 16, W by 1024) so the fp16 "lo" residuals stay in normal
# fp16 range; the resulting 2^18 logit scale is folded into the exp affine.
#
# x is transposed on the tensor engine per 128x128 chunk (fine-grained deps so
# the PE ramps up immediately); the softmax output is transposed via DMA xbar.
import sys

import numpy as np

for _p in ("/opt/trn_rl_repo",):
    if _p not in sys.path:
        sys.path.insert(0, _p)

import concourse.bass as bass
from concourse import bacc
import concourse.mybir as mybir
import concourse.tile as tile
from concourse.bass_utils import run_bass_kernel_spmd

F32 = mybir.dt.float32
F16 = mybir.dt.float16

P = 128
FD = 512  # matmul moving free dim (one fp32 PSUM bank)

SX = 16.0  # x / y pre-scale
SW = 1024.0  # W pre-scale
SLOG = SX * SX * SW  # net logit scale = 2**18
MASKC = -1.0e9 * SLOG

NPASS = 1  # split passes per logit matmul (1 = plain fp16; rel err ~2.3e-3, gate is 2e-2)

ALU = mybir.AluOpType
ACTF = mybir.ActivationFunctionType
AXIS = mybir.AxisListType


def build_nc(n=1024):
    """Build the per-core Bass program (SPMD: same program on all 8 cores)."""
    NT = n // P  # 128-tiles per dim
    NH = n // FD  # 512-halves per dim
    HC = NT // NH  # transposed chunks per half (4)

    nc = bacc.Bacc("TRN2", target_bir_lowering=False, debug=False)
    x_d = nc.dram_tensor("x", [n, n], F32, kind="ExternalInput")
    y_d = nc.dram_tensor("y", [n, n], F32, kind="ExternalInput")
    mask_d = nc.dram_tensor("mask", [n, n], F32, kind="ExternalInput")
    w_d = nc.dram_tensor("W", [n, n], F32, kind="ExternalInput")
    b_d = nc.dram_tensor("bvec", [1, n], F32, kind="ExternalInput")
    id_d = nc.dram_tensor("ident", [P, P], F32, kind="ExternalInput")
    out_d = nc.dram_tensor("out", [n, n], F32, kind="ExternalOutput")

    with tile.TileContext(nc) as tc:
        import contextlib

        ctx = contextlib.ExitStack()
        with ctx:
            persist = ctx.enter_context(tc.tile_pool(name="persist", bufs=1))
            ld = ctx.enter_context(tc.tile_pool(name="ld", bufs=3))
            epi = ctx.enter_context(tc.tile_pool(name="epi", bufs=2))
            small = ctx.enter_context(tc.tile_pool(name="small", bufs=4))
            psum = ctx.enter_context(tc.tile_pool(name="psum", bufs=7, space="PSUM"))
            psum_r = ctx.enter_context(
                tc.tile_pool(name="psum_r", bufs=1, space="PSUM")
            )
            dram = ctx.enter_context(
                tc.tile_pool(name="dram", bufs=1, space="DRAM")
            )

            # ---- persistent tensors (fp16 slabs are [P, NT, n] = 16KB/part) --
            x_hi = persist.tile([P, NT, n], F16, tag="x_hi")
            y_hi = persist.tile([P, NT, n], F16, tag="y_hi")
            y_lo = None
            if NPASS >= 2:
                y_lo = persist.tile([P, NT, n], F16, tag="slotD")
            gt_hi = persist.tile([P, NT, n], F16, tag="gt_hi")
            gt_lo = None
            if NPASS >= 2:
                gt_lo = persist.tile([P, NT, n], F16, tag="gt_lo")
            # transposed x, one slab per s-half: [P, kt, hc, P]
            xTh = [
                persist.tile(
                    [P, NT, HC, P], F16, tag=f"xTh_{h}", name=f"xTh_{h}"
                )
                for h in range(NH)
            ]
            xTl = None
            if NPASS >= 3:
                xTl = [
                    persist.tile(
                        [P, NT, HC, P], F16, tag=f"xTl_{h}", name=f"xTl_{h}"
                    )
                    for h in range(NH)
                ]

            ident = persist.tile([P, P], F32, tag="ident")
            nc.sync.dma_start(ident, id_d[:, :])
            # HAM warm-up: ~4.3us of dummy matmuls so the PE clock is at 8/8
            # before the first real transpose/matmul arrives.
            scratch = persist.tile([P, FD], F16, tag="scratch")
            nc.gpsimd.memset(scratch, 0.0)
            # fp32 ident@ident: depends only on the first sync-queue DMA, so
            # the PE clock is warming by ~2us
            wps = psum_r.tile([P, FD], F32, tag="rsx", name="warm_ps")
            for i in range(7):
                nc.tensor.matmul(
                    wps[:, 0:P], lhsT=ident, rhs=ident, start=(i == 0), stop=(i == 6)
                )
            rxs = persist.tile([P, NT], F32, tag="rxs")
            recip = [
                persist.tile([P, 1], F32, tag=f"recip{i}", name=f"recip{i}")
                for i in range(NT)
            ]
            et = [
                [
                    persist.tile(
                        [P, HC, P], F16, tag=f"et{i}_{h}", name=f"et{i}_{h}"
                    )
                    for h in range(NH)
                ]
                for i in range(NT)
            ]

            # ---- stage 0: load x, split, transpose chunks on PE -------------
            # x tiles 4..7 (only needed for the second g half) load after y so
            # y gets full DMA bandwidth during the ramp.
            def x_tile(it):
                xt = ld.tile([P, n], F32, tag="ld", name=f"xt{it}")
                nc.sync.dma_start(xt, x_d[P * it : P * (it + 1), :])
                # exact power-of-two pre-scale so every downstream op is a copy
                nc.vector.tensor_scalar_mul(xt, xt, SX)
                nc.vector.tensor_copy(x_hi[:, it, :], xt)
                h, hc = it // HC, it % HC
                for cb in range(NT // HC):
                    ptb = psum.tile(
                        [P, HC, P], F32, tag="mm", name=f"pt_{it}_{cb}"
                    )
                    for j in range(HC):
                        c = cb * HC + j
                        nc.tensor.transpose(
                            ptb[:, j, :], xt[:, P * c : P * (c + 1)], ident
                        )
                    dsth = xTh[h][:, cb * HC : (cb + 1) * HC, hc, :]
                    nc.vector.tensor_copy(dsth, ptb)
                    if NPASS >= 3:
                        nc.vector.scalar_tensor_tensor(
                            out=xTl[h][:, cb * HC : (cb + 1) * HC, hc, :],
                            in0=ptb,
                            scalar=1.0,
                            in1=dsth,
                            op0=ALU.mult,
                            op1=ALU.subtract,
                        )
                    hp = psum_r.tile(
                        [P, FD], F32, tag="rsx", name=f"ham_{it}_{cb}"
                    )
                    for i in range(2):
                        nc.tensor.matmul(
                            hp,
                            lhsT=scratch[:, 0:P],
                            rhs=scratch,
                            start=(i == 0),
                            stop=(i == 1),
                        )
                nc.vector.tensor_reduce(
                    rxs[:, it : it + 1], xt, axis=AXIS.X, op=ALU.add
                )

            for it in range(NT):
                x_tile(it)

            # ---- load y (SWDGE queue, concurrent with x on HWDGE), split ----
            for kt in range(NT):
                yt = ld.tile([P, n], F32, tag="ld")
                nc.gpsimd.dma_start(yt, y_d[P * kt : P * (kt + 1), :])
                nc.vector.tensor_scalar_mul(y_hi[:, kt, :], yt, SX)
                if NPASS >= 2:
                    nc.vector.scalar_tensor_tensor(
                        out=y_lo[:, kt, :],
                        in0=yt,
                        scalar=SX,
                        in1=y_hi[:, kt, :],
                        op0=ALU.mult,
                        op1=ALU.subtract,
                    )

            # bias term is added on DVE in the softmax epilogue:
            #   am += (SLOG/SX * b)[t] * (SX * rowsum_x)[s]
            # b is scaled fp32, broadcast across partitions via a DRAM bounce
            # (descriptor replication). Traced after the y loads so the
            # broadcast traffic does not delay the g-stage ramp.
            b_sb = persist.tile([1, n], F32, tag="b_sb")
            nc.gpsimd.dma_start(b_sb, b_d[:, :])
            nc.vector.tensor_scalar_mul(b_sb, b_sb, SLOG / SX)
            b_dr = dram.tile([1, n], F32, name="b_dr")
            nc.gpsimd.dma_start(b_dr[0:1, :], b_sb)
            b_bc = persist.tile([P, n], F32, tag="b_bc")
            bsrc = b_dr[0:1, :]
            nc.gpsimd.dma_start(
                b_bc[:, :],
                bass.AP(tensor=bsrc.tensor, offset=bsrc.offset,
                        ap=[[0, P], bsrc.ap[1]]),
            )

            # ---- g stage: gT[d, s] = SX^2 * sum_k x[s,k] y[k,d] -------------
            # sh-major so the first half only needs x tiles 0..HC-1 transposed.
            g_passes = [(y_hi, xTh)]
            if NPASS >= 2:
                g_passes.append((y_lo, xTh))
            if NPASS >= 3:
                g_passes.append((y_hi, xTl))
            nmm = NPASS * NT

            def g_epilogue(sh, dt, ps):
                nc.vector.tensor_copy(
                    gt_hi[:, dt, FD * sh : FD * (sh + 1)], ps
                )
                if NPASS >= 2:
                    nc.vector.scalar_tensor_tensor(
                        out=gt_lo[:, dt, FD * sh : FD * (sh + 1)],
                        in0=ps,
                        scalar=1.0,
                        in1=gt_hi[:, dt, FD * sh : FD * (sh + 1)],
                        op0=ALU.mult,
                        op1=ALU.subtract,
                    )

            def g_wavefront(sh, dts):
                # interleave groups across psum banks: each arriving y tile
                # unlocks len(dts)*NPASS matmuls so the PE FIFO never blocks
                # on a single group's kt ladder during the load phase
                pss = [
                    (dt, psum.tile([P, FD], F32, tag="mm", name=f"g{sh}_{dt}"))
                    for dt in dts
                ]
                cnt = dict.fromkeys(dts, 0)
                for kt in range(NT):
                    for dt, ps in pss:
                        for lhs, rhs in g_passes:
                            nc.tensor.matmul(
                                ps,
                                lhsT=lhs[:, kt, P * dt : P * (dt + 1)],
                                rhs=rhs[sh][:, kt, :, :],
                                start=(cnt[dt] == 0),
                                stop=(cnt[dt] == nmm - 1),
                            )
                            cnt[dt] += 1
                for dt, ps in pss:
                    g_epilogue(sh, dt, ps)

            def g_half(sh, wavefront=False):
                if wavefront:
                    g_wavefront(sh, list(range(4)))
                    rest = range(4, NT)
                else:
                    rest = range(NT)
                for dt in rest:
                    ps = psum.tile([P, FD], F32, tag="mm", name=f"g{sh}_{dt}")
                    idx = 0
                    for kt in range(NT):
                        for lhs, rhs in g_passes:
                            nc.tensor.matmul(
                                ps,
                                lhsT=lhs[:, kt, P * dt : P * (dt + 1)],
                                rhs=rhs[sh][:, kt, :, :],
                                start=(idx == 0),
                                stop=(idx == nmm - 1),
                            )
                            idx += 1
                    g_epilogue(sh, dt, ps)

            g_half(0, wavefront=True)
            for _sh in range(1, NH):
                g_half(_sh)

            # ---- load W (SWDGE), split (reuses x_lo / y_lo slots) -----------
            w_hi = persist.tile([P, NT, n], F16, tag="w_hi")
            w_lo = None
            if NPASS >= 3:
                w_lo = persist.tile([P, NT, n], F16, tag="w_lo", name="w_lo")
            for dt in range(NT):
                wt = ld.tile([P, n], F32, tag="ld")
                nc.gpsimd.dma_start(wt, w_d[P * dt : P * (dt + 1), :])
                nc.scalar.mul(w_hi[:, dt, :], wt, SW)
                if NPASS >= 3:
                    nc.vector.scalar_tensor_tensor(
                        out=w_lo[:, dt, :],
                        in0=wt,
                        scalar=SW,
                        in1=w_hi[:, dt, :],
                        op0=ALU.mult,
                        op1=ALU.subtract,
                    )

            # ---- a stage + softmax ------------------------------------------
            for st in range(NT):
                mk = ld.tile([P, n], F32, tag="ld")
                nc.sync.dma_start(mk, mask_d[P * st : P * (st + 1), :])
                am = epi.tile([P, n], F32, tag="am")
                for th in range(NH):
                    ps = psum.tile([P, FD], F32, tag="mm", name=f"a{st}_{th}")
                    idx = 0
                    a_passes = [(gt_hi, w_hi)]
                    if NPASS >= 2:
                        a_passes.append((gt_lo, w_hi))
                    if NPASS >= 3:
                        a_passes.append((gt_hi, w_lo))
                    for lhs, rhs in a_passes:
                        for dt in range(NT):
                            nc.tensor.matmul(
                                ps,
                                lhsT=lhs[:, dt, P * st : P * (st + 1)],
                                rhs=rhs[:, dt, FD * th : FD * (th + 1)],
                                start=(idx == 0),
                                stop=(idx == NPASS * NT - 1),
                            )
                            idx += 1
                    # masked scaled logits: am = mask*MASKC + psum
                    nc.vector.scalar_tensor_tensor(
                        out=am[:, FD * th : FD * (th + 1)],
                        in0=mk[:, FD * th : FD * (th + 1)],
                        scalar=MASKC,
                        in1=ps,
                        op0=ALU.mult,
                        op1=ALU.add,
                    )
                # am += b_bc[t] * rowsum_x[s]  (rank-1 bias on DVE)
                nc.vector.scalar_tensor_tensor(
                    out=am,
                    in0=b_bc,
                    scalar=rxs[:, st : st + 1],
                    in1=am,
                    op0=ALU.mult,
                    op1=ALU.add,
                )
                nm = small.tile([P, 1], F32, tag="nm")
                nc.vector.tensor_reduce(
                    nm, am, axis=AXIS.X, op=ALU.max, negate=True
                )
                nms = small.tile([P, 1], F32, tag="nms")
                nc.vector.tensor_scalar_mul(nms, nm, 1.0 / SLOG)
                eh = epi.tile([P, n], F16, tag="eh")
                rs = small.tile([P, 1], F32, tag="rs")
                nc.scalar.activation(
                    eh, am, ACTF.Exp, bias=nms, scale=1.0 / SLOG, accum_out=rs
                )
                nc.vector.reciprocal(recip[st], rs)
                for h in range(NH):
                    nc.scalar.dma_start_transpose(
                        et[st][h][:, :, :], eh[:, FD * h : FD * (h + 1)]
                    )

            # ---- out stage: out[s, e] = (e_hat @ x_hi) * recip / SX ---------
            for st in range(NT):
                # both halves interleaved: consecutive matmuls share lhsT and
                # group boundaries drop from every 8 matmuls to every 16
                opair = [
                    (h, psum.tile([P, FD], F32, tag="mm", name=f"o{st}_{h}"))
                    for h in range(NH)
                ]
                for tt in range(NT):
                    for h, ps in opair:
                        nc.tensor.matmul(
                            ps,
                            lhsT=et[st][tt // HC][:, tt % HC, :],
                            rhs=x_hi[:, tt, FD * h : FD * (h + 1)],
                            start=(tt == 0),
                            stop=(tt == NT - 1),
                        )
                for h, ps in opair:
                    ob = epi.tile([P, FD], F32, tag="ob")
                    nc.vector.tensor_scalar(
                        ob,
                        ps,
                        recip[st],
                        1.0 / SX,
                        ALU.mult,
                        ALU.mult,
                    )
                    nc.sync.dma_start(
                        out_d[P * st : P * (st + 1), FD * h : FD * (h + 1)], ob
                    )
    nc.compile()
    return nc


_NC_CACHE = {}


def _get_nc(n=1024):
    if n not in _NC_CACHE:
        _NC_CACHE[n] = build_nc(n)
    return _NC_CACHE[n]


def kernel(x, y, mask, W, b):
    """Full-input entry point: shard over batch across 8 cores, run, gather."""
    n = x.shape[-1]
    nc = _get_nc(n)
    Wc = np.ascontiguousarray(W, dtype=np.float32)
    bc = np.ascontiguousarray(np.asarray(b, dtype=np.float32).reshape(1, n))
    idc = np.eye(P, dtype=np.float32)
    in_maps = []
    for c in range(x.shape[0]):
        in_maps.append(
            {
                "x": np.ascontiguousarray(x[c], dtype=np.float32),
                "y": np.ascontiguousarray(y[c], dtype=np.float32),
                "mask": np.ascontiguousarray(mask[c], dtype=np.float32),
                "W": Wc,
                "bvec": bc,
                "ident": idc,
            }
        )
    res = run_bass_kernel_spmd(nc, in_maps, core_ids=list(range(len(in_maps))))
    return np.stack([r["out"] for r in res.results], axis=0)

